# revision 35
# baseline (speedup 1.0000x reference)
import math
import sys

sys.path.insert(0, "/opt/trn_rl_repo")

import ml_dtypes
import numpy as np

import concourse.bass as bass
import concourse.mybir as mybir
import concourse.tile as tile
from concourse import bacc
from concourse.bass_utils import run_bass_kernel_spmd

B, T, D, H, E = 4, 2048, 2048, 16, 128
N_CORES = 8
H_LOC = H // 2
P = 128
DT = D // P
KT = T // P
QW = 512
QC = T // QW
BF16 = mybir.dt.bfloat16
F32 = mybir.dt.float32
EXP_SCALE = 1.0 / math.sqrt(E)


def _build(repeat=1):
    nc = bacc.Bacc("TRN2", target_bir_lowering=False, debug=False,
                   num_devices=N_CORES)
    xT = nc.dram_tensor("xT", [D, T], BF16, kind="ExternalInput").ap()
    wqT = nc.dram_tensor("wqT", [H_LOC, D, E], BF16, kind="ExternalInput").ap()
    wkT = nc.dram_tensor("wkT", [H_LOC, D, E], BF16, kind="ExternalInput").ap()
    wvT = nc.dram_tensor("wvT", [H_LOC // 4, D, 4 * E], BF16,
                         kind="ExternalInput").ap()
    out = nc.dram_tensor("out", [H_LOC, E, T], F32, kind="ExternalOutput").ap()

    with tile.TileContext(nc) as tc:
        with (
            tc.tile_pool(name="xpool", bufs=1) as xpool,
            tc.tile_pool(name="wqk", bufs=2) as wqk,
            tc.tile_pool(name="wvp", bufs=1) as wvp,
            tc.tile_pool(name="qk", bufs=2) as qk,
            tc.tile_pool(name="vpool", bufs=2) as vpool,
            tc.tile_pool(name="ptp", bufs=6) as ptp,
            tc.tile_pool(name="outp", bufs=4) as outp,
            tc.tile_pool(name="smallp", bufs=3) as smallp,
            tc.tile_pool(name="dsum", bufs=2) as dsum,
            tc.tile_pool(name="onesp", bufs=1) as onesp,
            tc.tile_pool(name="stps", bufs=3, space="PSUM") as stps,
            tc.tile_pool(name="projps", bufs=2, space="PSUM") as projps,
            tc.tile_pool(name="ctps", bufs=2, space="PSUM") as ctps,
            tc.tile_pool(name="sumps", bufs=1, space="PSUM") as sumps,
        ):
            for _rep in range(repeat):
                _kernel_rep(tc, nc, xpool, wqk, wvp, qk, vpool, ptp, outp,
                            smallp, onesp, stps, projps, ctps, sumps,
                            xT, wqT, wkT, wvT, out, dsum)
    nc.compile()
    return nc


def _kernel_rep(tc, nc, xpool, wqk, wvp, qk, vpool, ptp, outp, smallp, onesp,
                stps, projps, ctps, sumps, xT, wqT, wkT, wvT, out, dsum):
    ones = onesp.tile([P, P], BF16)
    nc.vector.memset(ones[:], 1.0)

    def _load_w(h):
        wq_sb = wqk.tile([P, DT, E], BF16, tag="wq")
        nc.sync.dma_start(wq_sb[:], wqT[h].rearrange("(c p) e -> p c e", p=P))
        wk_sb = wqk.tile([P, DT, E], BF16, tag="wk")
        nc.sync.dma_start(wk_sb[:], wkT[h].rearrange("(c p) e -> p c e", p=P))
        return wq_sb, wk_sb

    xTr = xT.rearrange("(c p) t -> p c t", p=P)
    xs = []
    for c in range(DT):
        xt = xpool.tile([P, T], BF16, tag=f"x{c}")
        xs.append(xt)

    wq0_sb = wqk.tile([P, DT, E], BF16, tag="wq")
    nc.sync.dma_start(wq0_sb[:], wqT[0].rearrange("(c p) e -> p c e", p=P))
    nc.sync.dma_start(xs[0][:], xTr[:, 0, :])
    wk0_sb = wqk.tile([P, DT, E], BF16, tag="wk")
    nc.sync.dma_start(wk0_sb[:], wkT[0].rearrange("(c p) e -> p c e", p=P))
    for c in range(1, DT):
        nc.sync.dma_start(xs[c][:], xTr[:, c, :])
    w0 = (wq0_sb, wk0_sb)

    def _proj_qk(h, w=None, boost=False):
        wq_sb, wk_sb = w if w is not None else _load_w(h)
        qT = qk.tile([P, T], BF16, tag="qT")
        kT_sb = qk.tile([P, T], BF16, tag="kT")
        extra = [(ctps, "ct"), (sumps, "sum"), (stps, "st")]
        ci = 0
        for w_sb, oT in ((wq_sb, qT), (wk_sb, kT_sb)):
            for nt in range(QC):
                if boost and ci % 2 == 1 and ci // 2 < len(extra):
                    pool, tag = extra[ci // 2]
                else:
                    pool, tag = projps, "proj"
                ci += 1
                ps = pool.tile([P, QW], F32, tag=tag)
                for dt_i in range(DT):
                    nc.tensor.matmul(
                        ps[:], lhsT=w_sb[:, dt_i, :],
                        rhs=xs[dt_i][:, nt * QW:(nt + 1) * QW],
                        start=(dt_i == 0), stop=(dt_i == DT - 1))
                nc.vector.tensor_copy(oT[:, nt * QW:(nt + 1) * QW], ps[:])
        return qT, kT_sb

    def _attn(h, hi, qT, kT_sb, v_sb):
        for qc in range(QC):
            ct = ctps.tile([P, QW], F32, tag="ct")
            sm = sumps.tile([P, QW], F32, tag="sum")
            pts = [None] * KT

            def _ct(kt, ct=ct, pts=pts):
                nc.tensor.matmul(
                    ct[:], lhsT=v_sb[:, kt, hi * E:(hi + 1) * E],
                    rhs=pts[kt],
                    start=(kt == 0), stop=(kt == KT - 1))

            for kt in range(KT):
                st = stps.tile([P, QW], F32, tag="st")
                nc.tensor.matmul(
                    st[:], lhsT=kT_sb[:, kt * P:(kt + 1) * P],
                    rhs=qT[:, qc * QW:(qc + 1) * QW],
                    start=True, stop=True)
                pt = ptp.tile([P, QW], BF16, tag="pt")
                nc.scalar.activation(
                    pt[:], st[:], mybir.ActivationFunctionType.Exp,
                    scale=EXP_SCALE)
                pts[kt] = pt[:]
                if kt >= 1:
                    _ct(kt - 1)
                if kt % 4 == 3:
                    kp = kt // 4
                    d1 = dsum.tile([P, QW], BF16, tag="d1")
                    nc.vector.tensor_add(d1[:], pts[kt - 3], pts[kt - 2])
                    d2 = dsum.tile([P, QW], BF16, tag="d2")
                    nc.vector.tensor_add(d2[:], pts[kt - 1], pts[kt])
                    d12 = dsum.tile([P, QW], BF16, tag="d12")
                    nc.vector.tensor_add(d12[:], d1[:], d2[:])
                    nc.tensor.matmul(
                        sm[:], lhsT=ones[:], rhs=d12[:],
                        start=(kp == 0), stop=(kp == KT // 4 - 1))
            _ct(KT - 1)
            rec = smallp.tile([P, QW], F32, tag="rec")
            nc.vector.reciprocal(rec[:], sm[:])
            ot = outp.tile([P, QW], F32, tag="ot")
            nc.vector.tensor_mul(ot[:], ct[:], rec[:])
            nc.sync.dma_start(out[h, :, qc * QW:(qc + 1) * QW], ot[:])

    for quad in range(H_LOC // 4):
        qk0 = _proj_qk(4 * quad, w=w0 if quad == 0 else None,
                       boost=(quad == 0))

        wv_sb = wvp.tile([P, DT, 4 * E], BF16, tag="wv")
        wvr = wvT[quad].rearrange("(c p) e -> p c e", p=P)
        for c4 in range(0, DT, 4):
            nc.sync.dma_start(wv_sb[:, c4:c4 + 4, :], wvr[:, c4:c4 + 4, :])
        v_sb = vpool.tile([P, KT, 4 * E], BF16, tag="v")
        for kt in range(KT):
            ps = projps.tile([P, 4 * E], F32, tag="proj")
            for dt_i in range(DT):
                nc.tensor.matmul(
                    ps[:], lhsT=xs[dt_i][:, kt * P:(kt + 1) * P],
                    rhs=wv_sb[:, dt_i, :],
                    start=(dt_i == 0), stop=(dt_i == DT - 1))
            nc.vector.tensor_copy(v_sb[:, kt, :], ps[:])

        for hi in range(4):
            h = 4 * quad + hi
            qT, kT_sb = qk0 if hi == 0 else _proj_qk(h)
            _attn(h, hi, qT, kT_sb, v_sb)


_NC_CACHE = {}


def _get_nc():
    if "nc" not in _NC_CACHE:
        _NC_CACHE["nc"] = _build()
    return _NC_CACHE["nc"]


def _prep_in_maps(x, Wq, Wk, Wv):
    bf = ml_dtypes.bfloat16
    x16 = np.asarray(x).astype(bf)
    Wq16 = np.asarray(Wq).astype(bf)
    Wk16 = np.asarray(Wk).astype(bf)
    Wv16 = np.asarray(Wv).astype(bf)

    xT_by_b = [np.ascontiguousarray(x16[b].T) for b in range(B)]
    wq_by_g, wk_by_g, wv_by_g = [], [], []
    for g in range(2):
        sl = slice(g * H_LOC * E, (g + 1) * H_LOC * E)
        wq_by_g.append(np.ascontiguousarray(
            Wq16[sl].reshape(H_LOC, E, D).transpose(0, 2, 1)))
        wk_by_g.append(np.ascontiguousarray(
            Wk16[sl].reshape(H_LOC, E, D).transpose(0, 2, 1)))
        wv_by_g.append(np.ascontiguousarray(
            Wv16[sl].reshape(H_LOC // 4, 4, E, D)
            .transpose(0, 3, 1, 2).reshape(H_LOC // 4, D, 4 * E)))

    in_maps = []
    for c in range(N_CORES):
        b, g = divmod(c, 2)
        in_maps.append({
            "xT": xT_by_b[b],
            "wqT": wq_by_g[g],
            "wkT": wk_by_g[g],
            "wvT": wv_by_g[g],
        })
    return in_maps


def run_sharded(x, Wq, Wk, Wv, **spmd_kwargs):
    nc = _get_nc()
    in_maps = _prep_in_maps(x, Wq, Wk, Wv)
    res = run_bass_kernel_spmd(nc, in_maps, list(range(N_CORES)), **spmd_kwargs)
    full = np.empty((B, H, T, E), np.float32)
    for c in range(N_CORES):
        b, g = divmod(c, 2)
        oc = res.results[c]["out"]
        full[b, g * H_LOC:(g + 1) * H_LOC] = oc.transpose(0, 2, 1)
    return full, res


def kernel(x, Wq, Wk, Wv):
    full, _ = run_sharded(x, Wq, Wk, Wv)
    return full
